# revision 1
# baseline (speedup 1.0000x reference)
"""Memristor linear layer kernel for 8 TRN2 NeuronCores.

The reference memristor crossbar computation collapses algebraically to
    out = x @ weights.T + bias
(the G_OFF offsets cancel in the pos/neg column subtraction and the k_G /
k_I scale factors cancel exactly), so the kernel computes the plain linear
layer.

Precision: fp32 operands are split on host into bf16 hi + bf16 lo halves;
the device computes hi*hi + hi*lo + lo*hi with fp32 PSUM accumulation
(~4e-6 relative error vs 3e-7 for native fp32) at full bf16 PE rate.

Sharding: tensor-parallel over the 1024 output features -> 128 per core.
Each core receives x.T (replicated) and its W.T column shard, pre-packed
on host into the exact SBUF layout [128 partitions, k_tile, free] so
every DMA moves per-partition-contiguous rows at line rate. Each core
computes its out.T shard [128, 256] = W_shard @ x.T + bias accumulated
over 8 K-chunks of 128 in PSUM. Host concatenates and transposes back.

Schedule notes (from NTFF profiling on TRN2 under axon):
- The HWDGE rings drain in global issue order at ~280 GB/s, with ~1 us
  per-transfer completion latency, so transfers are staged in the exact
  order the matmul passes need them (wh | xh halves, wl, xl halves).
- The PE HAM clock gate needs ~3.4 us of sustained busy-ness to release
  (1.2 -> 2.4 GHz) and re-throttles after ~2 us of idle, so garbage
  warm-up matmuls run while DMAs stream and tiny filler matmuls are
  interleaved between compute passes to bridge DMA chase-stalls.
"""

import os

import numpy as np

BATCH = 256
SIZE_IN = 1024
SIZE_OUT = 1024
N_CORES = 8
O_SHARD = SIZE_OUT // N_CORES  # 128
K_TILES = SIZE_IN // 128  # 8

_STATE = {}


def _build():
    import concourse.bass as bass
    import concourse.tile as tile
    from concourse import bacc, mybir

    f32 = mybir.dt.float32
    bf16 = mybir.dt.bfloat16
    n_warm = int(os.environ.get("WARMUP_MM", "5"))

    nc = bacc.Bacc(None, target_bir_lowering=False)

    # All tensors pre-packed on host to [128, ..., free] (partition major)
    # so every DMA descriptor is a large per-partition-contiguous run.
    xh_d = nc.declare_dram_parameter("xh", [128, K_TILES, BATCH], bf16, isOutput=False)
    xl_d = nc.declare_dram_parameter("xl", [128, K_TILES, BATCH], bf16, isOutput=False)
    whl_d = nc.declare_dram_parameter(
        "whl", [128, 2, K_TILES, O_SHARD], bf16, isOutput=False
    )
    b_d = nc.declare_dram_parameter("bias", [O_SHARD, 1], f32, isOutput=False)
    out_d = nc.declare_dram_parameter("out", [O_SHARD, BATCH], f32, isOutput=True)

    with tile.TileContext(nc) as tc:
        with (
            tc.tile_pool(name="sbuf", bufs=1) as pool,
            tc.tile_pool(name="psum", bufs=1, space="PSUM") as psum_pool,
        ):
            xh_s = pool.tile([128, K_TILES, BATCH], bf16)
            xl_s = pool.tile([128, K_TILES, BATCH], bf16)
            whl_s = pool.tile([128, 2, K_TILES, O_SHARD], bf16)
            b_s = pool.tile([O_SHARD, 1], f32)
            o_s = pool.tile([O_SHARD, BATCH], f32)
            pt = psum_pool.tile([O_SHARD, BATCH], f32)

            # PE warm-up: garbage matmuls into a scratch PSUM bank so the
            # HAM clock-gate releases (1.2 -> 2.4 GHz) while DMAs stream.
            # A few big ones build the busy window, then small (~54 ns)
            # ones keep PE occupied at fine granularity until real data
            # lands; more small ones are interleaved between the compute
            # passes below so DMA chase-stalls can't re-throttle the PE.
            n_warm_small = int(os.environ.get("WARMUP_MM_SMALL", "30"))
            warm_in = pool.tile([128, 512], bf16)
            warm_ps = psum_pool.tile([128, 512], f32)
            nc.vector.memset(warm_in[:], 0.0)

            def warm_big(n):
                for _ in range(n):
                    nc.tensor.matmul(
                        warm_ps[:], warm_in[:, 0:128], warm_in[:], start=True,
                        stop=True,
                    )

            def warm_small(n):
                for _ in range(n):
                    nc.tensor.matmul(
                        warm_ps[:, 0:64], warm_in[:, 0:128], warm_in[:, 0:64],
                        start=True, stop=True,
                    )

            warm_big(n_warm)
            warm_small(n_warm_small)

            # Fine-grained transfers. Each engine issues its own queue in
            # program order and the HWDGE drains in global issue-time
            # order, so keep everything whose order matters on the scalar
            # ring; sync carries only the two wh halves issued up front.
            h = K_TILES // 2
            variant = os.environ.get("DMA_VARIANT", "min4")
            if variant == "minw":
                # weights hi+lo and x hi combined in ONE 1MB transfer
                # (8 KB/partition descriptors, one less transfer boundary)
                wx_d = nc.declare_dram_parameter(
                    "wx", [128, 4096], bf16, isOutput=False
                )
                wx_s = pool.tile([128, 4096], bf16)
                nc.sync.dma_start(out=wx_s[:], in_=wx_d[:])
                nc.scalar.dma_start(out=xl_s[:, 0:h, :], in_=xl_d[:, 0:h, :])
                nc.scalar.dma_start(out=xl_s[:, h:, :], in_=xl_d[:, h:, :])

                def wh_k(k):
                    return wx_s[:, k * 128 : (k + 1) * 128]

                def wl_k(k):
                    return wx_s[:, 1024 + k * 128 : 1024 + (k + 1) * 128]

                def xh_k(k):
                    return wx_s[:, 2048 + k * 256 : 2048 + (k + 1) * 256]

                ap_plan = []
                for k in range(K_TILES):
                    ap_plan.append((wh_k(k), xh_k(k)))
                    ap_plan.append((wl_k(k), xh_k(k)))
                    if k == h - 1:
                        ap_plan.append(None)
                ap_plan.append(None)
                ap_plan += [
                    (wh_k(k), xl_s[:, k, :]) for k in range(K_TILES)
                ]
                plan = None
            elif variant in ("min4", "min4b", "min3"):
                # Minimal transfer count: the kernel end is stream-bound,
                # so per-transfer overhead matters more than fine gating
                # (the PE has slack to absorb coarser chunks).
                nc.sync.dma_start(out=whl_s[:], in_=whl_d[:])
                nc.scalar.dma_start(out=xh_s[:], in_=xh_d[:])
                if variant == "min4":
                    nc.scalar.dma_start(out=xl_s[:, 0:h, :], in_=xl_d[:, 0:h, :])
                    nc.scalar.dma_start(out=xl_s[:, h:, :], in_=xl_d[:, h:, :])
                elif variant == "min4b":
                    # uneven split: tiny last transfer so only 2 matmuls
                    # remain after the stream ends
                    nc.scalar.dma_start(out=xl_s[:, 0:6, :], in_=xl_d[:, 0:6, :])
                    nc.scalar.dma_start(out=xl_s[:, 6:, :], in_=xl_d[:, 6:, :])
                else:
                    nc.scalar.dma_start(out=xl_s[:], in_=xl_d[:])
                plan = []
                for k in range(K_TILES):
                    plan.append((0, xh_s, k))
                    plan.append((1, xh_s, k))
                    if k == h - 1:
                        plan.append(None)
                plan.append(None)
                plan += [(0, xl_s, k) for k in range(K_TILES)]
            elif variant == "par":
                # balanced rings: sync 768KB, scalar 768KB — tests whether
                # the two HWDGE rings can drain concurrently
                nc.sync.dma_start(out=whl_s[:], in_=whl_d[:])
                nc.scalar.dma_start(out=xh_s[:, 0:h, :], in_=xh_d[:, 0:h, :])
                nc.scalar.dma_start(out=xh_s[:, h:, :], in_=xh_d[:, h:, :])
                nc.sync.dma_start(out=xl_s[:, 0:h, :], in_=xl_d[:, 0:h, :])
                nc.scalar.dma_start(out=xl_s[:, h:, :], in_=xl_d[:, h:, :])
                plan = (
                    [(0, xh_s, k) for k in range(h)]
                    + [(1, xh_s, k) for k in range(h)]
                    + [None]
                    + [(0, xh_s, k) for k in range(h, K_TILES)]
                    + [(1, xh_s, k) for k in range(h, K_TILES)]
                    + [None]
                    + [(0, xl_s, k) for k in range(K_TILES)]
                )
            elif variant == "whl1":
                # One early 512 KB weight transfer (hi+lo), then x hi and
                # x lo halves chase on the scalar ring. Both weight halves
                # are ready when the first x chunk lands, so the lo*hi
                # pass interleaves early and only hi*lo waits for x lo.
                nc.sync.dma_start(out=whl_s[:], in_=whl_d[:])
                nc.scalar.dma_start(out=xh_s[:, 0:h, :], in_=xh_d[:, 0:h, :])
                nc.scalar.dma_start(out=xh_s[:, h:, :], in_=xh_d[:, h:, :])
                nc.scalar.dma_start(out=xl_s[:, 0:h, :], in_=xl_d[:, 0:h, :])
                nc.scalar.dma_start(out=xl_s[:, h:, :], in_=xl_d[:, h:, :])
                plan = (
                    [(0, xh_s, k) for k in range(h)]
                    + [(1, xh_s, k) for k in range(h)]
                    + [None]
                    + [(0, xh_s, k) for k in range(h, K_TILES)]
                    + [(1, xh_s, k) for k in range(h, K_TILES)]
                    + [None]
                    + [(0, xl_s, k) for k in range(K_TILES)]
                )
            else:
                # wh | xh halves | wl | xl halves in need order
                nc.sync.dma_start(out=whl_s[:, 0, :, :], in_=whl_d[:, 0, :, :])
                nc.scalar.dma_start(out=xh_s[:, 0:h, :], in_=xh_d[:, 0:h, :])
                nc.scalar.dma_start(out=xh_s[:, h:, :], in_=xh_d[:, h:, :])
                nc.sync.dma_start(out=whl_s[:, 1, :, :], in_=whl_d[:, 1, :, :])
                nc.scalar.dma_start(out=xl_s[:, 0:h, :], in_=xl_d[:, 0:h, :])
                nc.scalar.dma_start(out=xl_s[:, h:, :], in_=xl_d[:, h:, :])
                plan = (
                    [(0, xh_s, k) for k in range(h)]
                    + [None]
                    + [(0, xh_s, k) for k in range(h, K_TILES)]
                    + [None]
                    + [(1, xh_s, k) for k in range(K_TILES)]
                    + [None]
                    + [(0, xl_s, k) for k in range(K_TILES)]
                )
            # bias: tiny transfer; by default on the scalar ring tail so
            # the gpsimd engine (slow SWDGE drain) stays completely idle
            if os.environ.get("BIAS_GPSIMD", "0") == "1":
                nc.gpsimd.dma_start(out=b_s[:], in_=b_d[:])
            else:
                nc.scalar.dma_start(out=b_s[:], in_=b_d[:])
            if plan is not None:
                ap_plan = [
                    (whl_s[:, p[0], p[2], :], p[1][:, p[2], :])
                    if p is not None
                    else None
                    for p in plan
                ]
            n_mm = len([p for p in ap_plan if p is not None])
            i = 0
            for p in ap_plan:
                if p is None:
                    warm_small(int(os.environ.get("WARMUP_MM_GAP", "8")))
                    continue
                nc.tensor.matmul(
                    pt[:],
                    p[0],
                    p[1],
                    start=(i == 0),
                    stop=(i == n_mm - 1),
                )
                i += 1

            # bias-add/copy in halves: the first out-half DMA issues while
            # the second half is still copying; halves ride both HWDGE
            # rings so the completion receipts (~1 us each to HBM) overlap
            hb = BATCH // 2
            if os.environ.get("TS_SPLIT", "1") == "1":
                nc.vector.tensor_scalar_add(
                    out=o_s[:, 0:hb], in0=pt[:, 0:hb], scalar1=b_s[:]
                )
                nc.sync.dma_start(out=out_d[:, 0:hb], in_=o_s[:, 0:hb])
                nc.vector.tensor_scalar_add(
                    out=o_s[:, hb:], in0=pt[:, hb:], scalar1=b_s[:]
                )
                nc.scalar.dma_start(out=out_d[:, hb:], in_=o_s[:, hb:])
            else:
                nc.vector.tensor_scalar_add(out=o_s[:], in0=pt[:], scalar1=b_s[:])
                nc.sync.dma_start(out=out_d[:, 0:hb], in_=o_s[:, 0:hb])
                nc.scalar.dma_start(out=out_d[:, hb:], in_=o_s[:, hb:])

    nc.compile()
    return nc


def _install_ntff_hook_shim():
    """The agent image's antenv lacks axon_hooks; recreate it so
    run_bass_kernel_spmd(trace=True) can capture NTFF profiles."""
    import sys
    import types

    if "antenv.axon_hooks" in sys.modules:
        return
    try:
        import antenv.axon_hooks  # noqa: F401  (real module exists)

        return
    except ImportError:
        pass
    mod = types.ModuleType("antenv.axon_hooks")
    mod._HOOK = None

    def set_axon_ntff_profile_hook(hook):
        mod._HOOK = hook

    def get_axon_ntff_profile_hook():
        return mod._HOOK

    mod.set_axon_ntff_profile_hook = set_axon_ntff_profile_hook
    mod.get_axon_ntff_profile_hook = get_axon_ntff_profile_hook
    sys.modules["antenv.axon_hooks"] = mod
    try:
        from trn_agent_boot.trn_boot import _ntff_profile_via_ctypes

        mod._HOOK = _ntff_profile_via_ctypes("/opt/axon/libaxon_pjrt.so")
    except Exception:
        pass


def _split_pack(a_t: np.ndarray, ncols: int):
    """[SIZE_IN, ncols] f32 -> two bf16 arrays packed as [128, K_TILES, ncols]."""
    import ml_dtypes

    hi = a_t.astype(ml_dtypes.bfloat16)
    lo = (a_t - hi.astype(np.float32)).astype(ml_dtypes.bfloat16)

    def pack(v):
        return np.ascontiguousarray(
            v.reshape(K_TILES, 128, ncols).transpose(1, 0, 2)
        )

    return pack(hi), pack(lo)


def _split_pack_w(w_t: np.ndarray):
    """[SIZE_IN, O_SHARD] f32 -> one bf16 array [128, 2, K_TILES, O_SHARD]
    holding the hi and lo halves contiguously per partition."""
    hi, lo = _split_pack(w_t, O_SHARD)
    return np.ascontiguousarray(np.stack([hi, lo], axis=1))


def kernel(x: np.ndarray, weights: np.ndarray, bias: np.ndarray) -> np.ndarray:
    from concourse.bass_utils import run_bass_kernel_spmd

    if "nc" not in _STATE:
        _STATE["nc"] = _build()
    nc = _STATE["nc"]

    x = np.asarray(x, dtype=np.float32)
    weights = np.asarray(weights, dtype=np.float32)
    bias = np.asarray(bias, dtype=np.float32)

    xt = np.ascontiguousarray(x.T)  # [SIZE_IN, BATCH] f32
    xh, xl = _split_pack(xt, BATCH)
    wt = np.ascontiguousarray(weights.T)  # [SIZE_IN, SIZE_OUT] f32

    minw = os.environ.get("DMA_VARIANT", "min4") == "minw"
    in_maps = []
    for c in range(N_CORES):
        sl = slice(c * O_SHARD, (c + 1) * O_SHARD)
        whl = _split_pack_w(np.ascontiguousarray(wt[:, sl]))
        m = {
            "xh": xh,
            "xl": xl,
            "whl": whl,
            "bias": np.ascontiguousarray(bias[sl]).reshape(O_SHARD, 1),
        }
        if minw:
            m["wx"] = np.ascontiguousarray(
                np.concatenate(
                    [whl.reshape(128, -1), xh.reshape(128, -1)], axis=1
                )
            )
        in_maps.append(m)

    # Always install the shim: if BASS_TRACE is set in the environment,
    # run_bass_kernel_spmd imports antenv.axon_hooks unconditionally and
    # would otherwise crash on images whose antenv lacks that module.
    _install_ntff_hook_shim()
    trace = os.environ.get("BASS_PROBLEM_TRACE", "0") == "1"
    res = run_bass_kernel_spmd(
        nc, in_maps, core_ids=list(range(N_CORES)), trace=trace
    )
    _STATE["last_results"] = res

    out_t = np.concatenate(
        [np.asarray(res.results[c]["out"]) for c in range(N_CORES)], axis=0
    )  # [SIZE_OUT, BATCH]
    return np.ascontiguousarray(out_t.T).astype(np.float32, copy=False)



# revision 2
# speedup vs baseline: 1.1268x; 1.1268x over previous
"""Memristor linear layer kernel for 8 TRN2 NeuronCores.

The reference memristor crossbar computation collapses algebraically to
    out = x @ weights.T + bias
(the G_OFF offsets cancel in the pos/neg column subtraction and the k_G /
k_I scale factors cancel exactly), so the kernel computes the plain linear
layer.

Precision: single-pass bf16 (hi halves only). Measured on the real
problem inputs: rel err 2.34e-3 vs the 2e-2 gate — an 8.5x margin.
(The previous 3-pass hi/lo split scheme reached 4.4e-6 but streamed 2x
the bytes; this kernel is DMA-stream-bound so bytes win.)

Sharding: tensor-parallel over the 1024 output features -> 128 per core.
Each core receives x.T (replicated, bf16, [128, 8, 256]) and its W.T
column shard ([128, 8, 128] bf16), pre-packed on host into the exact
SBUF layout so every DMA moves per-partition-contiguous rows. Each core
computes its out.T shard [128, 256] = W_shard @ x.T + bias accumulated
over 8 K-chunks of 128 in PSUM. Host concatenates and transposes back.

Schedule: w shard (256KB) on the sync HWDGE ring, x (512KB) chunked on
the scalar ring so matmuls chase the stream; garbage warm-up matmuls
keep the PE busy (HAM clock gate 1.2 -> 2.4 GHz) while DMAs stream.
Output is bias-added in halves riding both HWDGE rings so the ~1us
HBM-write receipts overlap.
"""

import os

import numpy as np

BATCH = 256
SIZE_IN = 1024
SIZE_OUT = 1024
N_CORES = 8
O_SHARD = SIZE_OUT // N_CORES  # 128
K_TILES = SIZE_IN // 128  # 8

_STATE = {}


def _env(name, default):
    return os.environ.get(name, default)


def _build():
    import concourse.bass as bass
    import concourse.tile as tile
    from concourse import bacc, mybir

    f32 = mybir.dt.float32
    bf16 = mybir.dt.bfloat16

    # --- knobs -----------------------------------------------------------
    # x k-tile chunking, comma-separated (sums to 8)
    x_chunks = [int(c) for c in _env("X_CHUNKS", "2,2,2,2").split(",")]
    assert sum(x_chunks) == K_TILES
    # which ring each x chunk rides: 'as' = alternate sync/scalar ...
    x_rings = _env("X_RINGS", "s" * len(x_chunks))  # 's'=scalar 'y'=sync 'g'=gpsimd
    assert len(x_rings) == len(x_chunks)
    n_warm_big = int(_env("WARM_BIG", "3"))
    n_warm_start = int(_env("WARM_START", "10"))
    n_warm_gap = int(_env("WARM_GAP", "8"))
    out_bf16 = _env("OUT_BF16", "0") == "1"
    ts_split = _env("TS_SPLIT", "1") == "1"
    # ---------------------------------------------------------------------

    nc = bacc.Bacc(None, target_bir_lowering=False)

    xh_d = nc.declare_dram_parameter("xh", [128, K_TILES, BATCH], bf16, isOutput=False)
    wh_d = nc.declare_dram_parameter("wh", [128, K_TILES, O_SHARD], bf16, isOutput=False)
    b_d = nc.declare_dram_parameter("bias", [O_SHARD, 1], f32, isOutput=False)
    out_dt = bf16 if out_bf16 else f32
    out_d = nc.declare_dram_parameter("out", [O_SHARD, BATCH], out_dt, isOutput=True)

    with tile.TileContext(nc) as tc:
        with (
            tc.tile_pool(name="sbuf", bufs=1) as pool,
            tc.tile_pool(name="psum", bufs=1, space="PSUM") as psum_pool,
        ):
            xh_s = pool.tile([128, K_TILES, BATCH], bf16)
            wh_s = pool.tile([128, K_TILES, O_SHARD], bf16)
            b_s = pool.tile([O_SHARD, 1], f32)
            o_s = pool.tile([O_SHARD, BATCH], out_dt)
            pt = psum_pool.tile([O_SHARD, BATCH], f32)

            # PE warm-up: garbage matmuls into a scratch PSUM bank so the
            # HAM clock-gate releases (1.2 -> 2.4 GHz) while DMAs stream;
            # small fillers bridge chunk-boundary stalls so the PE is at
            # speed for the exposed tail matmuls after the last x chunk.
            warm_in = pool.tile([128, 512], bf16)
            warm_ps = psum_pool.tile([128, 512], f32)
            nc.vector.memset(warm_in[:], 0.0)

            def warm_big(n):
                for _ in range(n):
                    nc.tensor.matmul(
                        warm_ps[:], warm_in[:, 0:128], warm_in[:], start=True,
                        stop=True,
                    )

            def warm_small(n):
                for _ in range(n):
                    nc.tensor.matmul(
                        warm_ps[:, 0:64], warm_in[:, 0:128], warm_in[:, 0:64],
                        start=True, stop=True,
                    )

            warm_big(n_warm_big)
            warm_small(n_warm_start)

            # DMA issue, in need order. w first (LDWEIGHTS gates every
            # matmul), then x chunks that the matmuls chase, bias last
            # (only needed by the final bias-add).
            nc.sync.dma_start(out=wh_s[:], in_=wh_d[:])
            k0 = 0
            ring = {"s": nc.scalar, "y": nc.sync, "g": nc.gpsimd}
            for ck, r in zip(x_chunks, x_rings):
                ring[r].dma_start(
                    out=xh_s[:, k0 : k0 + ck, :], in_=xh_d[:, k0 : k0 + ck, :]
                )
                k0 += ck
            nc.sync.dma_start(out=b_s[:], in_=b_d[:])

            # Matmul plan: accumulate the 8 k-tiles into PSUM, inserting
            # warm fillers at chunk boundaries (where the PE would stall
            # waiting for the next x chunk anyway).
            boundaries = set()
            acc = 0
            for ck in x_chunks[:-1]:
                acc += ck
                boundaries.add(acc)
            for k in range(K_TILES):
                if k in boundaries:
                    warm_small(n_warm_gap)
                nc.tensor.matmul(
                    pt[:],
                    wh_s[:, k, :],
                    xh_s[:, k, :],
                    start=(k == 0),
                    stop=(k == K_TILES - 1),
                )

            # bias-add/copy in halves: the first out-half DMA issues while
            # the second half is still copying; halves ride both HWDGE
            # rings so the ~1us HBM-write receipts overlap.
            hb = BATCH // 2
            if ts_split:
                nc.vector.tensor_scalar_add(
                    out=o_s[:, 0:hb], in0=pt[:, 0:hb], scalar1=b_s[:]
                )
                nc.sync.dma_start(out=out_d[:, 0:hb], in_=o_s[:, 0:hb])
                nc.vector.tensor_scalar_add(
                    out=o_s[:, hb:], in0=pt[:, hb:], scalar1=b_s[:]
                )
                nc.scalar.dma_start(out=out_d[:, hb:], in_=o_s[:, hb:])
            else:
                nc.vector.tensor_scalar_add(out=o_s[:], in0=pt[:], scalar1=b_s[:])
                nc.sync.dma_start(out=out_d[:, 0:hb], in_=o_s[:, 0:hb])
                nc.scalar.dma_start(out=out_d[:, hb:], in_=o_s[:, hb:])

    nc.compile()
    return nc


def _install_ntff_hook_shim():
    """The agent image's antenv lacks axon_hooks; recreate it so
    run_bass_kernel_spmd(trace=True) can capture NTFF profiles."""
    import sys
    import types

    if "antenv.axon_hooks" in sys.modules:
        return
    try:
        import antenv.axon_hooks  # noqa: F401  (real module exists)

        return
    except ImportError:
        pass
    mod = types.ModuleType("antenv.axon_hooks")
    mod._HOOK = None

    def set_axon_ntff_profile_hook(hook):
        mod._HOOK = hook

    def get_axon_ntff_profile_hook():
        return mod._HOOK

    mod.set_axon_ntff_profile_hook = set_axon_ntff_profile_hook
    mod.get_axon_ntff_profile_hook = get_axon_ntff_profile_hook
    sys.modules["antenv.axon_hooks"] = mod
    try:
        from trn_agent_boot.trn_boot import _ntff_profile_via_ctypes

        mod._HOOK = _ntff_profile_via_ctypes("/opt/axon/libaxon_pjrt.so")
    except Exception:
        pass


def _pack(a_t: np.ndarray, ncols: int) -> np.ndarray:
    """[SIZE_IN, ncols] f32 -> bf16 packed as [128, K_TILES, ncols]."""
    import ml_dtypes

    hi = a_t.astype(ml_dtypes.bfloat16)
    return np.ascontiguousarray(hi.reshape(K_TILES, 128, ncols).transpose(1, 0, 2))


def kernel(x: np.ndarray, weights: np.ndarray, bias: np.ndarray) -> np.ndarray:
    from concourse.bass_utils import run_bass_kernel_spmd

    if "nc" not in _STATE:
        _STATE["nc"] = _build()
    nc = _STATE["nc"]

    x = np.asarray(x, dtype=np.float32)
    weights = np.asarray(weights, dtype=np.float32)
    bias = np.asarray(bias, dtype=np.float32)

    xt = np.ascontiguousarray(x.T)  # [SIZE_IN, BATCH] f32
    xh = _pack(xt, BATCH)
    wt = np.ascontiguousarray(weights.T)  # [SIZE_IN, SIZE_OUT] f32

    in_maps = []
    for c in range(N_CORES):
        sl = slice(c * O_SHARD, (c + 1) * O_SHARD)
        in_maps.append(
            {
                "xh": xh,
                "wh": _pack(np.ascontiguousarray(wt[:, sl]), O_SHARD),
                "bias": np.ascontiguousarray(bias[sl]).reshape(O_SHARD, 1),
            }
        )

    # Always install the shim: if BASS_TRACE is set in the environment,
    # run_bass_kernel_spmd imports antenv.axon_hooks unconditionally and
    # would otherwise crash on images whose antenv lacks that module.
    _install_ntff_hook_shim()
    trace = os.environ.get("BASS_PROBLEM_TRACE", "0") == "1"
    res = run_bass_kernel_spmd(
        nc, in_maps, core_ids=list(range(N_CORES)), trace=trace
    )
    _STATE["last_results"] = res

    out_t = np.concatenate(
        [
            np.asarray(res.results[c]["out"]).astype(np.float32)
            for c in range(N_CORES)
        ],
        axis=0,
    )  # [SIZE_OUT, BATCH]
    return np.ascontiguousarray(out_t.T).astype(np.float32, copy=False)


# revision 3
# speedup vs baseline: 1.1445x; 1.0157x over previous
"""Memristor linear layer kernel for 8 TRN2 NeuronCores.

The reference memristor crossbar computation collapses algebraically to
    out = x @ weights.T + bias
(the G_OFF offsets cancel in the pos/neg column subtraction and the k_G /
k_I scale factors cancel exactly), so the kernel computes the plain linear
layer.

Precision: single-pass bf16. Measured on the real problem inputs:
rel err 2.9e-3 (incl. bf16 output rounding) vs the 2e-2 gate.

Sharding: tensor-parallel over the 1024 output features -> 128 per core.
Each core gets x.T (replicated bf16 [128, 8, 256]) and its W.T column
shard ([128, 8, 128] bf16); computes out.T shard [128, 256] accumulated
over 8 K-tiles of 128 in PSUM; host concatenates and transposes back.

Schedule (packet-count driven — HWDGE drains ~flat per packet, packets
= min(descriptor, 4KB), descriptor = per-partition contiguous run, so
every transfer keeps >=1KB per partition and the bias rides as a single
512B descriptor [1,128] broadcast across partitions by a K=1 matmul):
  sync ring:   bias row (1 pkt) | wh (128x2KB) | x k4:6 (128x1KB) | out
  scalar ring: x k0:4 (128x2KB) | x k6:8 (128x1KB)
  PE: bias-broadcast MM, then k0..7 accumulate, chasing the stream.
No warm-up matmuls: this kernel is too short for the HAM clock gate to
ever release (PE stays at 1.2 GHz regardless), and queued fillers delay
the tail matmuls once data catches up.
"""

import os

import numpy as np

BATCH = 256
SIZE_IN = 1024
SIZE_OUT = 1024
N_CORES = 8
O_SHARD = SIZE_OUT // N_CORES  # 128
K_TILES = SIZE_IN // 128  # 8

_STATE = {}


def _build():
    import concourse.bass as bass
    import concourse.tile as tile
    from concourse import bacc, mybir

    f32 = mybir.dt.float32
    bf16 = mybir.dt.bfloat16

    out_bf16 = os.environ.get("OUT_BF16", "1") == "1"
    out_dt = bf16 if out_bf16 else f32

    nc = bacc.Bacc(None, target_bir_lowering=False)

    brow_d = nc.declare_dram_parameter("brow", [1, O_SHARD], f32, isOutput=False)
    wh_d = nc.declare_dram_parameter("wh", [128, K_TILES, O_SHARD], bf16, isOutput=False)
    x_d = nc.declare_dram_parameter("xh", [128, K_TILES, BATCH], bf16, isOutput=False)
    out_d = nc.declare_dram_parameter("out", [O_SHARD, BATCH], out_dt, isOutput=True)

    with tile.TileContext(nc) as tc:
        with (
            tc.tile_pool(name="sbuf", bufs=1) as pool,
            tc.tile_pool(name="psum", bufs=1, space="PSUM") as psum_pool,
        ):
            brow_s = pool.tile([1, O_SHARD], f32)
            ones_s = pool.tile([1, 1], f32)
            wh_s = pool.tile([128, K_TILES, O_SHARD], bf16)
            x_s = pool.tile([128, K_TILES, BATCH], bf16)
            b_s = pool.tile([O_SHARD, 1], f32)
            o_s = pool.tile([O_SHARD, BATCH], out_dt)
            pt = psum_pool.tile([O_SHARD, BATCH], f32)
            pb = psum_pool.tile([O_SHARD, 1], f32)

            nc.vector.memset(ones_s[:], 1.0)

            # DMA issue order = need order; both HWDGE rings pull
            # concurrently. wh gates every LDWEIGHTS so it leads sync.
            nc.sync.dma_start(out=brow_s[:], in_=brow_d[:])
            nc.sync.dma_start(out=wh_s[:], in_=wh_d[:])
            nc.scalar.dma_start(out=x_s[:, 0:4, :], in_=x_d[:, 0:4, :])
            nc.sync.dma_start(out=x_s[:, 4:6, :], in_=x_d[:, 4:6, :])
            nc.scalar.dma_start(out=x_s[:, 6:8, :], in_=x_d[:, 6:8, :])

            # Broadcast bias [1,128] -> [128,1] with a K=1 matmul
            # (brow^T @ 1), then park it in SBUF for the bias-add.
            nc.tensor.matmul(pb[:], brow_s[:], ones_s[:], start=True, stop=True)
            nc.scalar.copy(out=b_s[:], in_=pb[:])

            for k in range(K_TILES):
                nc.tensor.matmul(
                    pt[:],
                    wh_s[:, k, :],
                    x_s[:, k, :],
                    start=(k == 0),
                    stop=(k == K_TILES - 1),
                )

            nc.vector.tensor_scalar_add(out=o_s[:], in0=pt[:], scalar1=b_s[:])
            nc.sync.dma_start(out=out_d[:], in_=o_s[:])

    nc.compile()
    return nc


def _install_ntff_hook_shim():
    """The agent image's antenv lacks axon_hooks; recreate it so
    run_bass_kernel_spmd(trace=True) can capture NTFF profiles."""
    import sys
    import types

    if "antenv.axon_hooks" in sys.modules:
        return
    try:
        import antenv.axon_hooks  # noqa: F401  (real module exists)

        return
    except ImportError:
        pass
    mod = types.ModuleType("antenv.axon_hooks")
    mod._HOOK = None

    def set_axon_ntff_profile_hook(hook):
        mod._HOOK = hook

    def get_axon_ntff_profile_hook():
        return mod._HOOK

    mod.set_axon_ntff_profile_hook = set_axon_ntff_profile_hook
    mod.get_axon_ntff_profile_hook = get_axon_ntff_profile_hook
    sys.modules["antenv.axon_hooks"] = mod
    try:
        from trn_agent_boot.trn_boot import _ntff_profile_via_ctypes

        mod._HOOK = _ntff_profile_via_ctypes("/opt/axon/libaxon_pjrt.so")
    except Exception:
        pass


def _pack(a_t: np.ndarray, ncols: int) -> np.ndarray:
    """[SIZE_IN, ncols] f32 -> bf16 packed as [128, K_TILES, ncols]."""
    import ml_dtypes

    hi = a_t.astype(ml_dtypes.bfloat16)
    return np.ascontiguousarray(hi.reshape(K_TILES, 128, ncols).transpose(1, 0, 2))


def kernel(x: np.ndarray, weights: np.ndarray, bias: np.ndarray) -> np.ndarray:
    from concourse.bass_utils import run_bass_kernel_spmd

    if "nc" not in _STATE:
        _STATE["nc"] = _build()
    nc = _STATE["nc"]

    x = np.asarray(x, dtype=np.float32)
    weights = np.asarray(weights, dtype=np.float32)
    bias = np.asarray(bias, dtype=np.float32)

    xt = np.ascontiguousarray(x.T)  # [SIZE_IN, BATCH] f32
    xh = _pack(xt, BATCH)
    wt = np.ascontiguousarray(weights.T)  # [SIZE_IN, SIZE_OUT] f32

    in_maps = []
    for c in range(N_CORES):
        sl = slice(c * O_SHARD, (c + 1) * O_SHARD)
        in_maps.append(
            {
                "xh": xh,
                "wh": _pack(np.ascontiguousarray(wt[:, sl]), O_SHARD),
                "brow": np.ascontiguousarray(bias[sl]).reshape(1, O_SHARD),
            }
        )

    # Always install the shim: if BASS_TRACE is set in the environment,
    # run_bass_kernel_spmd imports antenv.axon_hooks unconditionally and
    # would otherwise crash on images whose antenv lacks that module.
    _install_ntff_hook_shim()
    trace = os.environ.get("BASS_PROBLEM_TRACE", "0") == "1"
    res = run_bass_kernel_spmd(
        nc, in_maps, core_ids=list(range(N_CORES)), trace=trace
    )
    _STATE["last_results"] = res

    out_t = np.concatenate(
        [
            np.asarray(res.results[c]["out"]).astype(np.float32)
            for c in range(N_CORES)
        ],
        axis=0,
    )  # [SIZE_OUT, BATCH]
    return np.ascontiguousarray(out_t.T).astype(np.float32, copy=False)


# revision 4
# speedup vs baseline: 1.1495x; 1.0044x over previous
"""Memristor linear layer kernel for 8 TRN2 NeuronCores.

The reference memristor crossbar computation collapses algebraically to
    out = x @ weights.T + bias
(the G_OFF offsets cancel in the pos/neg column subtraction and the k_G /
k_I scale factors cancel exactly), so the kernel computes the plain
linear layer. The bias-add (a [1024]-vector broadcast over 256 rows) is
folded into the host-side unshard pass; the device computes x @ W.T.

Precision: single-pass bf16, bf16 output. Measured on the real problem
inputs: rel err 2.9e-3 vs the 2e-2 gate.

Sharding: tensor-parallel over the 1024 output features -> 128 per core.
Each core gets x.T (replicated bf16 [128, 8, 256]) and its W.T column
shard ([128, 8, 128] bf16); computes out.T shard [128, 256] accumulated
over 8 K-tiles of 128 in PSUM; host concatenates, adds bias, transposes.

Schedule (packet-count driven: HWDGE+SDMA move ~one packet slot per
~5-10ns combined across both rings at >=2KB descriptors, descriptor =
per-partition contiguous run; every transfer here keeps >=1KB per
partition):
  sync ring:   wh (128x2KB) | x k4:8 (128x2KB) | out (128x512B bf16)
  scalar ring: x k0:4 (128x2KB)
  PE:          k0..k7 accumulate into PSUM, chasing the two x chunks
  DVE+ACT:     copy PSUM->SBUF in halves on two engines in parallel
No warm-up matmuls: the kernel is too short for the HAM clock gate to
release (PE stays at 1.2 GHz regardless), and queued fillers delay the
tail matmuls once data catches up.
"""

import os

import numpy as np

BATCH = 256
SIZE_IN = 1024
SIZE_OUT = 1024
N_CORES = 8
O_SHARD = SIZE_OUT // N_CORES  # 128
K_TILES = SIZE_IN // 128  # 8

_STATE = {}


def _build():
    import concourse.bass as bass
    import concourse.tile as tile
    from concourse import bacc, mybir

    f32 = mybir.dt.float32
    bf16 = mybir.dt.bfloat16

    out_bf16 = os.environ.get("OUT_BF16", "1") == "1"
    out_dt = bf16 if out_bf16 else f32

    nc = bacc.Bacc(None, target_bir_lowering=False)

    wh_d = nc.declare_dram_parameter("wh", [128, K_TILES, O_SHARD], bf16, isOutput=False)
    x_d = nc.declare_dram_parameter("xh", [128, K_TILES, BATCH], bf16, isOutput=False)
    out_d = nc.declare_dram_parameter("out", [O_SHARD, BATCH], out_dt, isOutput=True)

    with tile.TileContext(nc) as tc:
        with (
            tc.tile_pool(name="sbuf", bufs=1) as pool,
            tc.tile_pool(name="psum", bufs=1, space="PSUM") as psum_pool,
        ):
            wh_s = pool.tile([128, K_TILES, O_SHARD], bf16)
            x_s = pool.tile([128, K_TILES, BATCH], bf16)
            o_s = pool.tile([O_SHARD, BATCH], out_dt)
            pt = psum_pool.tile([O_SHARD, BATCH], f32)

            # DMA issue order = need order; both HWDGE rings pull
            # concurrently and share the 16 SDMA engines. wh gates every
            # LDWEIGHTS so it leads the sync ring with no issue queued
            # ahead of it; the x halves chase on both rings.
            nc.sync.dma_start(out=wh_s[:], in_=wh_d[:])
            nc.scalar.dma_start(out=x_s[:, 0:4, :], in_=x_d[:, 0:4, :])
            nc.sync.dma_start(out=x_s[:, 4:8, :], in_=x_d[:, 4:8, :])

            for k in range(K_TILES):
                nc.tensor.matmul(
                    pt[:],
                    wh_s[:, k, :],
                    x_s[:, k, :],
                    start=(k == 0),
                    stop=(k == K_TILES - 1),
                )

            # PSUM -> SBUF copy in halves on two engines concurrently
            # (DMA cannot read PSUM); the out DMA waits on both.
            hb = BATCH // 2
            nc.vector.tensor_scalar_add(
                out=o_s[:, 0:hb], in0=pt[:, 0:hb], scalar1=0.0
            )
            nc.scalar.copy(out=o_s[:, hb:], in_=pt[:, hb:])
            nc.sync.dma_start(out=out_d[:], in_=o_s[:])

    nc.compile()
    return nc


def _install_ntff_hook_shim():
    """The agent image's antenv lacks axon_hooks; recreate it so
    run_bass_kernel_spmd(trace=True) can capture NTFF profiles."""
    import sys
    import types

    if "antenv.axon_hooks" in sys.modules:
        return
    try:
        import antenv.axon_hooks  # noqa: F401  (real module exists)

        return
    except ImportError:
        pass
    mod = types.ModuleType("antenv.axon_hooks")
    mod._HOOK = None

    def set_axon_ntff_profile_hook(hook):
        mod._HOOK = hook

    def get_axon_ntff_profile_hook():
        return mod._HOOK

    mod.set_axon_ntff_profile_hook = set_axon_ntff_profile_hook
    mod.get_axon_ntff_profile_hook = get_axon_ntff_profile_hook
    sys.modules["antenv.axon_hooks"] = mod
    try:
        from trn_agent_boot.trn_boot import _ntff_profile_via_ctypes

        mod._HOOK = _ntff_profile_via_ctypes("/opt/axon/libaxon_pjrt.so")
    except Exception:
        pass


def _pack(a_t: np.ndarray, ncols: int) -> np.ndarray:
    """[SIZE_IN, ncols] f32 -> bf16 packed as [128, K_TILES, ncols]."""
    import ml_dtypes

    hi = a_t.astype(ml_dtypes.bfloat16)
    return np.ascontiguousarray(hi.reshape(K_TILES, 128, ncols).transpose(1, 0, 2))


def kernel(x: np.ndarray, weights: np.ndarray, bias: np.ndarray) -> np.ndarray:
    from concourse.bass_utils import run_bass_kernel_spmd

    if "nc" not in _STATE:
        _STATE["nc"] = _build()
    nc = _STATE["nc"]

    x = np.asarray(x, dtype=np.float32)
    weights = np.asarray(weights, dtype=np.float32)
    bias = np.asarray(bias, dtype=np.float32)

    xt = np.ascontiguousarray(x.T)  # [SIZE_IN, BATCH] f32
    xh = _pack(xt, BATCH)
    wt = np.ascontiguousarray(weights.T)  # [SIZE_IN, SIZE_OUT] f32

    in_maps = []
    for c in range(N_CORES):
        sl = slice(c * O_SHARD, (c + 1) * O_SHARD)
        in_maps.append(
            {
                "xh": xh,
                "wh": _pack(np.ascontiguousarray(wt[:, sl]), O_SHARD),
            }
        )

    # Always install the shim: if BASS_TRACE is set in the environment,
    # run_bass_kernel_spmd imports antenv.axon_hooks unconditionally and
    # would otherwise crash on images whose antenv lacks that module.
    _install_ntff_hook_shim()
    trace = os.environ.get("BASS_PROBLEM_TRACE", "0") == "1"
    res = run_bass_kernel_spmd(
        nc, in_maps, core_ids=list(range(N_CORES)), trace=trace
    )
    _STATE["last_results"] = res

    out_t = np.concatenate(
        [
            np.asarray(res.results[c]["out"]).astype(np.float32)
            for c in range(N_CORES)
        ],
        axis=0,
    )  # [SIZE_OUT, BATCH]
    # bias-add folded into the host unshard (broadcast over batch)
    return np.ascontiguousarray(out_t.T + bias[None, :]).astype(
        np.float32, copy=False
    )


# revision 5
# speedup vs baseline: 1.1850x; 1.0309x over previous
"""Memristor linear layer kernel for 8 TRN2 NeuronCores.

The reference memristor crossbar computation collapses algebraically to
    out = x @ weights.T + bias
(the G_OFF offsets cancel in the pos/neg column subtraction and the k_G /
k_I scale factors cancel exactly), so the kernel computes the plain
linear layer. The bias-add (a [1024]-vector broadcast over 256 rows) is
folded into the host-side unshard pass; the device computes x @ W.T.

Precision: single-pass bf16, bf16 output. Measured on the real problem
inputs: rel err 2.9e-3 vs the 2e-2 gate.

Sharding: tensor-parallel over the 1024 output features -> 128 per core.
Each core gets x.T (replicated bf16 [128, 8, 256]) and its W.T column
shard ([128, 8, 128] bf16); computes out.T shard [128, 256] accumulated
over 8 K-tiles of 128 in PSUM; host concatenates, adds bias, transposes.

Schedule (packet-count driven: HWDGE+SDMA move ~one packet slot per
~5-10ns combined across both rings at >=2KB descriptors, descriptor =
per-partition contiguous run; every transfer here keeps >=1KB per
partition):
  sync ring:   wh (128x2KB) | x k4:8 (128x2KB) | out (128x512B bf16)
  scalar ring: x k0:4 (128x2KB)
  PE:          k0..k7 accumulate into PSUM, chasing the two x chunks
  DVE+ACT:     copy PSUM->SBUF in halves on two engines in parallel
No warm-up matmuls: the kernel is too short for the HAM clock gate to
release (PE stays at 1.2 GHz regardless), and queued fillers delay the
tail matmuls once data catches up.
"""

import os

import numpy as np

BATCH = 256
SIZE_IN = 1024
SIZE_OUT = 1024
N_CORES = 8
O_SHARD = SIZE_OUT // N_CORES  # 128
K_TILES = SIZE_IN // 128  # 8

_STATE = {}


def _build():
    import concourse.bass as bass
    import concourse.tile as tile
    from concourse import bacc, mybir

    f32 = mybir.dt.float32
    bf16 = mybir.dt.bfloat16

    out_bf16 = os.environ.get("OUT_BF16", "1") == "1"
    out_dt = bf16 if out_bf16 else f32

    nc = bacc.Bacc(None, target_bir_lowering=False)

    wh_d = nc.declare_dram_parameter("wh", [128, K_TILES, O_SHARD], bf16, isOutput=False)
    x_d = nc.declare_dram_parameter("xh", [128, K_TILES, BATCH], bf16, isOutput=False)
    out_d = nc.declare_dram_parameter("out", [O_SHARD, BATCH], out_dt, isOutput=True)

    with tile.TileContext(nc) as tc:
        with (
            tc.tile_pool(name="sbuf", bufs=1) as pool,
            tc.tile_pool(name="psum", bufs=1, space="PSUM") as psum_pool,
        ):
            wh_s = pool.tile([128, K_TILES, O_SHARD], bf16)
            x_s = pool.tile([128, K_TILES, BATCH], bf16)
            o_s = pool.tile([O_SHARD, BATCH], out_dt)
            pt = psum_pool.tile([O_SHARD, BATCH], f32)

            # DMA issue order = need order; both HWDGE rings pull
            # concurrently and share the 16 SDMA engines (~340 GB/s
            # combined at 2KB descriptors). wh + x k0:4 gate the first
            # matmul so they lead one ring each; the x tail chunks chase
            # behind them in ring-FIFO order. The activation engine is
            # never touched (its ACT_TABLE_LOAD preamble would delay the
            # scalar ring's first transfer by ~1us).
            nc.sync.dma_start(out=wh_s[:], in_=wh_d[:])
            nc.scalar.dma_start(out=x_s[:, 0:4, :], in_=x_d[:, 0:4, :])
            nc.sync.dma_start(out=x_s[:, 4:6, :], in_=x_d[:, 4:6, :])
            nc.scalar.dma_start(out=x_s[:, 6:8, :], in_=x_d[:, 6:8, :])

            for k in range(K_TILES):
                nc.tensor.matmul(
                    pt[:],
                    wh_s[:, k, :],
                    x_s[:, k, :],
                    start=(k == 0),
                    stop=(k == K_TILES - 1),
                )

            # PSUM -> SBUF copy (DMA cannot read PSUM), then one out DMA
            # on the scalar ring (it drains small packets faster and its
            # engine is idle by now).
            nc.vector.tensor_scalar_add(out=o_s[:], in0=pt[:], scalar1=0.0)
            nc.scalar.dma_start(out=out_d[:], in_=o_s[:])

    nc.compile()
    return nc


def _install_ntff_hook_shim():
    """The agent image's antenv lacks axon_hooks; recreate it so
    run_bass_kernel_spmd(trace=True) can capture NTFF profiles."""
    import sys
    import types

    if "antenv.axon_hooks" in sys.modules:
        return
    try:
        import antenv.axon_hooks  # noqa: F401  (real module exists)

        return
    except ImportError:
        pass
    mod = types.ModuleType("antenv.axon_hooks")
    mod._HOOK = None

    def set_axon_ntff_profile_hook(hook):
        mod._HOOK = hook

    def get_axon_ntff_profile_hook():
        return mod._HOOK

    mod.set_axon_ntff_profile_hook = set_axon_ntff_profile_hook
    mod.get_axon_ntff_profile_hook = get_axon_ntff_profile_hook
    sys.modules["antenv.axon_hooks"] = mod
    try:
        from trn_agent_boot.trn_boot import _ntff_profile_via_ctypes

        mod._HOOK = _ntff_profile_via_ctypes("/opt/axon/libaxon_pjrt.so")
    except Exception:
        pass


def _pack(a_t: np.ndarray, ncols: int) -> np.ndarray:
    """[SIZE_IN, ncols] f32 -> bf16 packed as [128, K_TILES, ncols]."""
    import ml_dtypes

    hi = a_t.astype(ml_dtypes.bfloat16)
    return np.ascontiguousarray(hi.reshape(K_TILES, 128, ncols).transpose(1, 0, 2))


def kernel(x: np.ndarray, weights: np.ndarray, bias: np.ndarray) -> np.ndarray:
    from concourse.bass_utils import run_bass_kernel_spmd

    if "nc" not in _STATE:
        _STATE["nc"] = _build()
    nc = _STATE["nc"]

    x = np.asarray(x, dtype=np.float32)
    weights = np.asarray(weights, dtype=np.float32)
    bias = np.asarray(bias, dtype=np.float32)

    xt = np.ascontiguousarray(x.T)  # [SIZE_IN, BATCH] f32
    xh = _pack(xt, BATCH)
    wt = np.ascontiguousarray(weights.T)  # [SIZE_IN, SIZE_OUT] f32

    in_maps = []
    for c in range(N_CORES):
        sl = slice(c * O_SHARD, (c + 1) * O_SHARD)
        in_maps.append(
            {
                "xh": xh,
                "wh": _pack(np.ascontiguousarray(wt[:, sl]), O_SHARD),
            }
        )

    # Always install the shim: if BASS_TRACE is set in the environment,
    # run_bass_kernel_spmd imports antenv.axon_hooks unconditionally and
    # would otherwise crash on images whose antenv lacks that module.
    _install_ntff_hook_shim()
    trace = os.environ.get("BASS_PROBLEM_TRACE", "0") == "1"
    res = run_bass_kernel_spmd(
        nc, in_maps, core_ids=list(range(N_CORES)), trace=trace
    )
    _STATE["last_results"] = res

    out_t = np.concatenate(
        [
            np.asarray(res.results[c]["out"]).astype(np.float32)
            for c in range(N_CORES)
        ],
        axis=0,
    )  # [SIZE_OUT, BATCH]
    # bias-add folded into the host unshard (broadcast over batch)
    return np.ascontiguousarray(out_t.T + bias[None, :]).astype(
        np.float32, copy=False
    )
